# revision 29
# baseline (speedup 1.0000x reference)
"""Block-sparse linear y = x @ W^T + b on 8 Trainium2 NeuronCores.

x: [8192, 4096] f32, W: [4096, 4096] f32 (block-sparse mask already applied),
b: [4096] f32 -> y: [8192, 4096] f32.

Sharding: 2 row-halves of x  x  4 out-feature quarters of W (8 cores).
Each core computes y_shard[4096, 1024] = x_half @ W_quarter^T + b_quarter.

Mixed precision K-split: first KF16 of the contraction in fp16 (1 cyc/row),
remaining KF8 in fp8-e4m3 DoubleRow matmuls (K=256 per MM, 0.5 cyc/row).
Rel err ~1.8e-2 (fp8 tail dominates), under the 2e-2 gate.
W-shards resident in SBUF; PSUM accumulates the full K per (m-tile, n-half).
"""

import contextlib

import numpy as np
import ml_dtypes
import jax
from jax.sharding import Mesh, NamedSharding, PartitionSpec
from jax.experimental.shard_map import shard_map

import concourse.tile as tile
from concourse import bacc, mybir
from concourse.bass2jax import (
    install_neuronx_cc_hook,
    partition_id_tensor,
    _bass_exec_p,
)

P = 128
K = 4096          # contraction (in_features)
N_ROWS = 8192     # x rows
D_OUT = 4096      # out_features
R_SHARDS = 2      # row shards
C_SHARDS = 4      # out-feature shards
MC = N_ROWS // R_SHARDS    # 4096 rows per core
NC_ = D_OUT // C_SHARDS    # 1024 out features per core
MT = MC // P               # 32 row-tiles

import os
KF16 = int(os.environ.get("BSL_KF16", "2816"))  # fp16 contraction span
KT16 = KF16 // P           # fp16 k-tiles
KF8 = K - KF16             # fp8 contraction span
KT8 = KF8 // (2 * P)       # DoubleRow steps (K=256 each)

F32 = mybir.dt.float32
F16 = mybir.dt.float16
F8 = mybir.dt.float8e4
NP_F8 = ml_dtypes.float8_e4m3

_CACHE = {}


def _build_nc(repeats=1):
    nc = bacc.Bacc("TRN2", target_bir_lowering=False)
    xt_d = nc.declare_dram_parameter("xt", [KF16, MC], F16, isOutput=False).ap()
    wt_d = nc.declare_dram_parameter("wt", [KF16, NC_], F16, isOutput=False).ap()
    x8_d = w8_d = None
    if KT8:
        x8_d = nc.declare_dram_parameter(
            "x8", [MT * P, KT8 * 2 * P], F8, isOutput=False
        ).ap()
        w8_d = nc.declare_dram_parameter(
            "w8", [P, KT8 * 2 * NC_], F8, isOutput=False
        ).ap()
    b_d = nc.declare_dram_parameter("bias", [P, NC_], F32, isOutput=False).ap()
    y_d = nc.declare_dram_parameter("y", [MC, NC_], F32, isOutput=True).ap()

    with tile.TileContext(nc) as tc:
        with (
            tc.tile_pool(name="wpool", bufs=1) as wpool,
            tc.tile_pool(name="bpool", bufs=1) as bpool,
            tc.tile_pool(name="xpool", bufs=5) as xpool,
            tc.tile_pool(name="opool", bufs=3) as opool,
            tc.tile_pool(name="psum", bufs=4, space="PSUM") as psum,
        ):
            # resident fp16 weights [p, kt, NC_]; split the load per k-tile so
            # early matmuls can start before the whole shard arrives
            wt_sb = wpool.tile([P, KT16, NC_], F16)
            wt_src = wt_d.rearrange("(j p) n -> p j n", p=P)
            for j in range(KT16):
                nc.sync.dma_start(out=wt_sb[:, j, :], in_=wt_src[:, j, :])
            w8_sb = x8_sb = None
            if KT8:
                # resident fp8 weights, DoubleRow layout [p, d, nh, h, q] so the
                # moving-operand slice [p, h, q] is a contiguous 1024B block
                w8_sb = wpool.tile([P, KT8, 2, 2, 512], F8)
                nc.sync.dma_start(
                    out=w8_sb[:],
                    in_=w8_d.rearrange(
                        "p (d nh h q) -> p d nh h q", d=KT8, nh=2, h=2
                    ),
                )
                # resident fp8 x, DoubleRow layout [p, m, d, h, q]
                x8_sb = wpool.tile([P, MT, KT8, 2, P], F8)
                x8_src = x8_d.rearrange(
                    "(m p) (d h q) -> p m d h q", p=P, d=KT8, h=2
                )
                for m in range(MT):
                    nc.sync.dma_start(out=x8_sb[:, m], in_=x8_src[:, m])
            b_sb = bpool.tile([P, NC_], F32)
            nc.sync.dma_start(out=b_sb[:], in_=b_d[:])

            rep_ctx = (
                tc.For_i(0, repeats, 1, hint_engines=(mybir.EngineType.PE,))
                if repeats > 1
                else contextlib.nullcontext()
            )
            with rep_ctx:
                _emit_body(
                    nc, tc, xpool, opool, psum, xt_d, y_d, wt_sb, w8_sb, x8_sb, b_sb
                )
    nc.compile()
    return nc


def _emit_body(nc, tc, xpool, opool, psum, xt_d, y_d, wt_sb, w8_sb, x8_sb, b_sb):
    DR = mybir.MatmulPerfMode.DoubleRow
    for m in range(MT):
        xt_sb = xpool.tile([P, KT16, P], F16)
        nc.sync.dma_start(
            out=xt_sb[:],
            in_=xt_d[:, m * P : (m + 1) * P].rearrange("(j p) f -> p j f", p=P),
        )
        o_sb = opool.tile([P, NC_], F32)
        if os.environ.get("BSL_CHAIN16"):
            # experiment: split each 32-MM chain into two 16-MM chains on two
            # banks, recombine on DVE (isolates chain-boundary cost)
            for n in range(2):
                psa = psum.tile([P, 512], F32, name="psa")
                psb = psum.tile([P, 512], F32, name="psb")
                half = KT16 // 2
                for j in range(half):
                    nc.tensor.matmul(
                        psa[:],
                        lhsT=xt_sb[:, j, :],
                        rhs=wt_sb[:, j, n * 512 : (n + 1) * 512],
                        start=(j == 0),
                        stop=(j == half - 1),
                    )
                for j in range(half, KT16):
                    nc.tensor.matmul(
                        psb[:],
                        lhsT=xt_sb[:, j, :],
                        rhs=wt_sb[:, j, n * 512 : (n + 1) * 512],
                        start=(j == half),
                        stop=(j == KT16 - 1),
                    )
                osl = o_sb[:, n * 512 : (n + 1) * 512]
                nc.vector.tensor_add(
                    out=osl, in0=psa[:], in1=b_sb[:, n * 512 : (n + 1) * 512]
                )
                nc.vector.tensor_add(out=osl, in0=psb[:], in1=osl)
            nc.sync.dma_start(out=y_d[m * P : (m + 1) * P, :], in_=o_sb[:])
            continue
        for n in range(2):
            ps = psum.tile([P, 512], F32)
            # fp16 span
            for j in range(KT16):
                nc.tensor.matmul(
                    ps[:],
                    lhsT=xt_sb[:, j, :],
                    rhs=wt_sb[:, j, n * 512 : (n + 1) * 512],
                    start=(j == 0),
                    stop=(not KT8 and j == KT16 - 1),
                )
            # fp8 DoubleRow span: K=256 per step
            for d in range(KT8):
                nc.tensor.matmul(
                    ps[:],
                    lhsT=x8_sb[:, m, d],
                    rhs=w8_sb[:, d, n],
                    start=False,
                    stop=(d == KT8 - 1),
                    perf_mode=DR,
                )
            nc.vector.tensor_add(
                out=o_sb[:, n * 512 : (n + 1) * 512],
                in0=ps[:],
                in1=b_sb[:, n * 512 : (n + 1) * 512],
            )
        nc.sync.dma_start(out=y_d[m * P : (m + 1) * P, :], in_=o_sb[:])


def _get_runner(repeats=1):
    """Build (once) a jitted 8-core executable: concat inputs -> concat outputs."""
    key = ("runner", repeats)
    if key in _CACHE:
        return _CACHE[key]

    install_neuronx_cc_hook()
    nc = _build_nc(repeats)

    partition_name = (
        nc.partition_id_tensor.name if nc.partition_id_tensor else None
    )
    in_names = []
    out_names = []
    out_avals = []
    out_shapes = []
    for alloc in nc.m.functions[0].allocations:
        if not isinstance(alloc, mybir.MemoryLocationSet):
            continue
        name = alloc.memorylocations[0].name
        if alloc.kind == "ExternalInput":
            if name != partition_name:
                in_names.append(name)
        elif alloc.kind == "ExternalOutput":
            shape = tuple(alloc.tensor_shape)
            out_names.append(name)
            out_shapes.append(shape)
            out_avals.append(
                jax.core.ShapedArray(shape, mybir.dt.np(alloc.dtype))
            )
    n_params = len(in_names)
    # outputs are passed as (non-donated) zero operands after the inputs
    all_names = in_names + out_names
    if partition_name is not None:
        all_names = all_names + [partition_name]

    def _body(*args):
        operands = list(args)
        if partition_name is not None:
            operands.append(partition_id_tensor())
        outs = _bass_exec_p.bind(
            *operands,
            out_avals=tuple(out_avals),
            in_names=tuple(all_names),
            out_names=tuple(out_names),
            lowering_input_output_aliases=(),
            sim_require_finite=True,
            sim_require_nnan=True,
            nc=nc,
        )
        return tuple(outs)

    devices = jax.devices()[:8]
    mesh = Mesh(np.asarray(devices), ("core",))
    n_outs = len(out_names)
    sharded = jax.jit(
        shard_map(
            _body,
            mesh=mesh,
            in_specs=(PartitionSpec("core"),) * (n_params + n_outs),
            out_specs=(PartitionSpec("core"),) * n_outs,
            check_rep=False,
        ),
        keep_unused=True,
    )
    runner = {
        "fn": sharded,
        "in_names": in_names,
        "out_names": out_names,
        "out_shapes": out_shapes,
        "mesh": mesh,
        "devices": devices,
    }
    _CACHE[key] = runner
    return runner


def _sharded_input(r, per_core):
    """Build a global sharded array from 8 per-core shards without a host concat."""
    sh = NamedSharding(r["mesh"], PartitionSpec("core"))
    shape = per_core[0].shape
    shards = [
        jax.device_put(np.ascontiguousarray(a), d)
        for a, d in zip(per_core, r["devices"])
    ]
    return jax.make_array_from_single_device_arrays(
        (8 * shape[0], *shape[1:]), sh, shards
    )


def _run_cores(in_maps, repeats=1):
    """in_maps: list of 8 dicts name->np.ndarray. Returns list of 8 output dicts."""
    r = _get_runner(repeats)
    concat_in = [
        _sharded_input(r, [np.asarray(m[name]) for m in in_maps])
        for name in r["in_names"]
    ]
    concat_zeros = [
        _sharded_input(r, [np.zeros(s, np.float32)] * 8) for s in r["out_shapes"]
    ]
    out_arrs = r["fn"](*concat_in, *concat_zeros)
    outs = []
    for c in range(8):
        outs.append(
            {
                name: np.asarray(out_arrs[i]).reshape(8, *r["out_shapes"][i])[c]
                for i, name in enumerate(r["out_names"])
            }
        )
    return outs


def _pack_x8(xt8):
    """xt8: [KF8, m] fp8 (k-major) -> [(m_tile p), (d h q)] DoubleRow layout."""
    m = xt8.shape[1]
    a = xt8.reshape(KT8, 2, P, m // P, P)        # d h p mt q
    a = a.transpose(3, 2, 0, 1, 4)               # mt p d h q
    return np.ascontiguousarray(a.reshape(m, KT8 * 2 * P))


def _pack_w8(wt8):
    """wt8: [KF8, n] fp8 (k-major) -> [p, (d nh h q)] DoubleRow layout."""
    n = wt8.shape[1]
    a = wt8.reshape(KT8, 2, P, n // 512, 512)    # d h p nh q
    a = a.transpose(2, 0, 3, 1, 4)               # p d nh h q
    return np.ascontiguousarray(a.reshape(P, KT8 * 2 * n))


def _make_in_maps(x, weight, bias):
    xf = np.asarray(x, dtype=np.float32).T
    wf = np.asarray(weight, dtype=np.float32).T
    xt = np.ascontiguousarray(xf[:KF16].astype(np.float16))
    wt = np.ascontiguousarray(wf[:KF16].astype(np.float16))
    xt8 = np.clip(xf[KF16:], -240, 240).astype(NP_F8)
    wt8 = np.clip(wf[KF16:], -240, 240).astype(NP_F8)
    bias = np.asarray(bias, dtype=np.float32)
    in_maps = []
    if KT8:
        x8_halves = [
            _pack_x8(xt8[:, h * MC : (h + 1) * MC]) for h in range(R_SHARDS)
        ]
        w8_quarters = [
            _pack_w8(wt8[:, q * NC_ : (q + 1) * NC_]) for q in range(C_SHARDS)
        ]
    for i in range(8):
        h, q = divmod(i, C_SHARDS)
        m = {
            "xt": xt[:, h * MC : (h + 1) * MC],
            "wt": wt[:, q * NC_ : (q + 1) * NC_],
            "bias": np.broadcast_to(bias[q * NC_ : (q + 1) * NC_], (P, NC_)),
        }
        if KT8:
            m["x8"] = x8_halves[h]
            m["w8"] = w8_quarters[q]
        in_maps.append(m)
    return in_maps


def kernel(x, weight, bias):
    in_maps = _make_in_maps(x, weight, bias)
    outs = _run_cores(in_maps)
    y = np.empty((N_ROWS, D_OUT), dtype=np.float32)
    for i in range(8):
        h, q = divmod(i, C_SHARDS)
        y[h * MC : (h + 1) * MC, q * NC_ : (q + 1) * NC_] = outs[i]["y"]
    return y


# revision 32
# speedup vs baseline: 1.2336x; 1.2336x over previous
"""Block-sparse linear y = x @ W^T + b on 8 Trainium2 NeuronCores.

x: [8192, 4096] f32, W: [4096, 4096] f32 (block-sparse mask already applied),
b: [4096] f32 -> y: [8192, 4096] f32.

Sharding: 2 row-halves of x  x  4 out-feature quarters of W (8 cores).
Each core computes y_shard[4096, 1024] = x_half @ W_quarter^T + b_quarter.

Each core computes with fp16 matmuls (fp32 PSUM accumulation, ~2.6e-4 rel
err), W-shard resident in SBUF, accumulating over 32 k-tiles in PSUM with
maximal 32-matmul accumulation chains (shorter chains measurably stall the
PE on this target). An optional fp8-e4m3 DoubleRow K-split tail exists
behind BSL_KF16 but measures slower than fp16 here, so the default runs
pure fp16 over the full contraction.
"""

import contextlib

import numpy as np
import ml_dtypes
import jax
from jax.sharding import Mesh, NamedSharding, PartitionSpec
from jax.experimental.shard_map import shard_map

import concourse.tile as tile
from concourse import bacc, mybir
from concourse.bass2jax import (
    install_neuronx_cc_hook,
    partition_id_tensor,
    _bass_exec_p,
)

P = 128
K = 4096          # contraction (in_features)
N_ROWS = 8192     # x rows
D_OUT = 4096      # out_features
R_SHARDS = 2      # row shards
C_SHARDS = 4      # out-feature shards
MC = N_ROWS // R_SHARDS    # 4096 rows per core
NC_ = D_OUT // C_SHARDS    # 1024 out features per core
MT = MC // P               # 32 row-tiles

import os
KF16 = int(os.environ.get("BSL_KF16", "4096"))  # fp16 contraction span
KT16 = KF16 // P           # fp16 k-tiles
KF8 = K - KF16             # fp8 contraction span
KT8 = KF8 // (2 * P)       # DoubleRow steps (K=256 each)

F32 = mybir.dt.float32
F16 = mybir.dt.float16
F8 = mybir.dt.float8e4
NP_F8 = ml_dtypes.float8_e4m3

_CACHE = {}


def _build_nc(repeats=1):
    nc = bacc.Bacc("TRN2", target_bir_lowering=False)
    xt_d = nc.declare_dram_parameter("xt", [KF16, MC], F16, isOutput=False).ap()
    wt_d = nc.declare_dram_parameter("wt", [KF16, NC_], F16, isOutput=False).ap()
    x8_d = w8_d = None
    if KT8:
        x8_d = nc.declare_dram_parameter(
            "x8", [MT * P, KT8 * 2 * P], F8, isOutput=False
        ).ap()
        w8_d = nc.declare_dram_parameter(
            "w8", [P, KT8 * 2 * NC_], F8, isOutput=False
        ).ap()
    b_d = nc.declare_dram_parameter("bias", [P, NC_], F32, isOutput=False).ap()
    y_d = nc.declare_dram_parameter("y", [MC, NC_], F32, isOutput=True).ap()

    with tile.TileContext(nc) as tc:
        with (
            tc.tile_pool(name="wpool", bufs=1) as wpool,
            tc.tile_pool(name="bpool", bufs=1) as bpool,
            tc.tile_pool(name="xpool", bufs=6) as xpool,
            tc.tile_pool(name="opool", bufs=4) as opool,
            tc.tile_pool(name="psum", bufs=6, space="PSUM") as psum,
        ):
            # resident fp16 weights [p, kt, NC_]; split the load per k-tile so
            # early matmuls can start before the whole shard arrives
            wt_sb = wpool.tile([P, KT16, NC_], F16)
            wt_src = wt_d.rearrange("(j p) n -> p j n", p=P)
            for j in range(KT16):
                nc.sync.dma_start(out=wt_sb[:, j, :], in_=wt_src[:, j, :])
            w8_sb = x8_sb = None
            if KT8:
                # resident fp8 weights, DoubleRow layout [p, d, nh, h, q] so the
                # moving-operand slice [p, h, q] is a contiguous 1024B block
                w8_sb = wpool.tile([P, KT8, 2, 2, 512], F8)
                nc.sync.dma_start(
                    out=w8_sb[:],
                    in_=w8_d.rearrange(
                        "p (d nh h q) -> p d nh h q", d=KT8, nh=2, h=2
                    ),
                )
                # resident fp8 x, DoubleRow layout [p, m, d, h, q]
                x8_sb = wpool.tile([P, MT, KT8, 2, P], F8)
                x8_src = x8_d.rearrange(
                    "(m p) (d h q) -> p m d h q", p=P, d=KT8, h=2
                )
                for m in range(MT):
                    nc.sync.dma_start(out=x8_sb[:, m], in_=x8_src[:, m])
            b_sb = bpool.tile([P, NC_], F32)
            nc.sync.dma_start(out=b_sb[:], in_=b_d[:])

            rep_ctx = (
                tc.For_i(0, repeats, 1, hint_engines=(mybir.EngineType.PE,))
                if repeats > 1
                else contextlib.nullcontext()
            )
            with rep_ctx:
                _emit_body(
                    nc, tc, xpool, opool, psum, xt_d, y_d, wt_sb, w8_sb, x8_sb, b_sb
                )
    nc.compile()
    return nc


def _emit_body(nc, tc, xpool, opool, psum, xt_d, y_d, wt_sb, w8_sb, x8_sb, b_sb):
    DR = mybir.MatmulPerfMode.DoubleRow
    for m in range(MT):
        xt_sb = xpool.tile([P, KT16, P], F16)
        nc.sync.dma_start(
            out=xt_sb[:],
            in_=xt_d[:, m * P : (m + 1) * P].rearrange("(j p) f -> p j f", p=P),
        )
        o_sb = opool.tile([P, NC_], F32)
        if os.environ.get("BSL_CHAIN16"):
            # experiment: split each 32-MM chain into two 16-MM chains on two
            # banks, recombine on DVE (isolates chain-boundary cost)
            for n in range(2):
                psa = psum.tile([P, 512], F32, name="psa")
                psb = psum.tile([P, 512], F32, name="psb")
                half = KT16 // 2
                for j in range(half):
                    nc.tensor.matmul(
                        psa[:],
                        lhsT=xt_sb[:, j, :],
                        rhs=wt_sb[:, j, n * 512 : (n + 1) * 512],
                        start=(j == 0),
                        stop=(j == half - 1),
                    )
                for j in range(half, KT16):
                    nc.tensor.matmul(
                        psb[:],
                        lhsT=xt_sb[:, j, :],
                        rhs=wt_sb[:, j, n * 512 : (n + 1) * 512],
                        start=(j == half),
                        stop=(j == KT16 - 1),
                    )
                osl = o_sb[:, n * 512 : (n + 1) * 512]
                nc.vector.tensor_add(
                    out=osl, in0=psa[:], in1=b_sb[:, n * 512 : (n + 1) * 512]
                )
                nc.vector.tensor_add(out=osl, in0=psb[:], in1=osl)
            nc.sync.dma_start(out=y_d[m * P : (m + 1) * P, :], in_=o_sb[:])
            continue
        for n in range(2):
            ps = psum.tile([P, 512], F32)
            # fp16 span
            for j in range(KT16):
                nc.tensor.matmul(
                    ps[:],
                    lhsT=xt_sb[:, j, :],
                    rhs=wt_sb[:, j, n * 512 : (n + 1) * 512],
                    start=(j == 0),
                    stop=(not KT8 and j == KT16 - 1),
                )
            # fp8 DoubleRow span: K=256 per step
            for d in range(KT8):
                nc.tensor.matmul(
                    ps[:],
                    lhsT=x8_sb[:, m, d],
                    rhs=w8_sb[:, d, n],
                    start=False,
                    stop=(d == KT8 - 1),
                    perf_mode=DR,
                )
            nc.vector.tensor_add(
                out=o_sb[:, n * 512 : (n + 1) * 512],
                in0=ps[:],
                in1=b_sb[:, n * 512 : (n + 1) * 512],
            )
        nc.sync.dma_start(out=y_d[m * P : (m + 1) * P, :], in_=o_sb[:])


def _get_runner(repeats=1):
    """Build (once) a jitted 8-core executable: concat inputs -> concat outputs."""
    key = ("runner", repeats)
    if key in _CACHE:
        return _CACHE[key]

    install_neuronx_cc_hook()
    nc = _build_nc(repeats)

    partition_name = (
        nc.partition_id_tensor.name if nc.partition_id_tensor else None
    )
    in_names = []
    out_names = []
    out_avals = []
    out_shapes = []
    for alloc in nc.m.functions[0].allocations:
        if not isinstance(alloc, mybir.MemoryLocationSet):
            continue
        name = alloc.memorylocations[0].name
        if alloc.kind == "ExternalInput":
            if name != partition_name:
                in_names.append(name)
        elif alloc.kind == "ExternalOutput":
            shape = tuple(alloc.tensor_shape)
            out_names.append(name)
            out_shapes.append(shape)
            out_avals.append(
                jax.core.ShapedArray(shape, mybir.dt.np(alloc.dtype))
            )
    n_params = len(in_names)
    # outputs are passed as (non-donated) zero operands after the inputs
    all_names = in_names + out_names
    if partition_name is not None:
        all_names = all_names + [partition_name]

    def _body(*args):
        operands = list(args)
        if partition_name is not None:
            operands.append(partition_id_tensor())
        outs = _bass_exec_p.bind(
            *operands,
            out_avals=tuple(out_avals),
            in_names=tuple(all_names),
            out_names=tuple(out_names),
            lowering_input_output_aliases=(),
            sim_require_finite=True,
            sim_require_nnan=True,
            nc=nc,
        )
        return tuple(outs)

    devices = jax.devices()[:8]
    mesh = Mesh(np.asarray(devices), ("core",))
    n_outs = len(out_names)
    sharded = jax.jit(
        shard_map(
            _body,
            mesh=mesh,
            in_specs=(PartitionSpec("core"),) * (n_params + n_outs),
            out_specs=(PartitionSpec("core"),) * n_outs,
            check_rep=False,
        ),
        keep_unused=True,
    )
    runner = {
        "fn": sharded,
        "in_names": in_names,
        "out_names": out_names,
        "out_shapes": out_shapes,
        "mesh": mesh,
        "devices": devices,
    }
    _CACHE[key] = runner
    return runner


def _sharded_input(r, per_core):
    """Build a global sharded array from 8 per-core shards without a host concat."""
    sh = NamedSharding(r["mesh"], PartitionSpec("core"))
    shape = per_core[0].shape
    shards = [
        jax.device_put(np.ascontiguousarray(a), d)
        for a, d in zip(per_core, r["devices"])
    ]
    return jax.make_array_from_single_device_arrays(
        (8 * shape[0], *shape[1:]), sh, shards
    )


def _run_cores(in_maps, repeats=1):
    """in_maps: list of 8 dicts name->np.ndarray. Returns list of 8 output dicts."""
    r = _get_runner(repeats)
    concat_in = [
        _sharded_input(r, [np.asarray(m[name]) for m in in_maps])
        for name in r["in_names"]
    ]
    concat_zeros = [
        _sharded_input(r, [np.zeros(s, np.float32)] * 8) for s in r["out_shapes"]
    ]
    out_arrs = r["fn"](*concat_in, *concat_zeros)
    outs = []
    for c in range(8):
        outs.append(
            {
                name: np.asarray(out_arrs[i]).reshape(8, *r["out_shapes"][i])[c]
                for i, name in enumerate(r["out_names"])
            }
        )
    return outs


def _pack_x8(xt8):
    """xt8: [KF8, m] fp8 (k-major) -> [(m_tile p), (d h q)] DoubleRow layout."""
    m = xt8.shape[1]
    a = xt8.reshape(KT8, 2, P, m // P, P)        # d h p mt q
    a = a.transpose(3, 2, 0, 1, 4)               # mt p d h q
    return np.ascontiguousarray(a.reshape(m, KT8 * 2 * P))


def _pack_w8(wt8):
    """wt8: [KF8, n] fp8 (k-major) -> [p, (d nh h q)] DoubleRow layout."""
    n = wt8.shape[1]
    a = wt8.reshape(KT8, 2, P, n // 512, 512)    # d h p nh q
    a = a.transpose(2, 0, 3, 1, 4)               # p d nh h q
    return np.ascontiguousarray(a.reshape(P, KT8 * 2 * n))


def _make_in_maps(x, weight, bias):
    xf = np.asarray(x, dtype=np.float32).T
    wf = np.asarray(weight, dtype=np.float32).T
    xt = np.ascontiguousarray(xf[:KF16].astype(np.float16))
    wt = np.ascontiguousarray(wf[:KF16].astype(np.float16))
    xt8 = np.clip(xf[KF16:], -240, 240).astype(NP_F8)
    wt8 = np.clip(wf[KF16:], -240, 240).astype(NP_F8)
    bias = np.asarray(bias, dtype=np.float32)
    in_maps = []
    if KT8:
        x8_halves = [
            _pack_x8(xt8[:, h * MC : (h + 1) * MC]) for h in range(R_SHARDS)
        ]
        w8_quarters = [
            _pack_w8(wt8[:, q * NC_ : (q + 1) * NC_]) for q in range(C_SHARDS)
        ]
    for i in range(8):
        h, q = divmod(i, C_SHARDS)
        m = {
            "xt": xt[:, h * MC : (h + 1) * MC],
            "wt": wt[:, q * NC_ : (q + 1) * NC_],
            "bias": np.broadcast_to(bias[q * NC_ : (q + 1) * NC_], (P, NC_)),
        }
        if KT8:
            m["x8"] = x8_halves[h]
            m["w8"] = w8_quarters[q]
        in_maps.append(m)
    return in_maps


def kernel(x, weight, bias):
    in_maps = _make_in_maps(x, weight, bias)
    outs = _run_cores(in_maps)
    y = np.empty((N_ROWS, D_OUT), dtype=np.float32)
    for i in range(8):
        h, q = divmod(i, C_SHARDS)
        y[h * MC : (h + 1) * MC, q * NC_ : (q + 1) * NC_] = outs[i]["y"]
    return y
